# revision 39
# baseline (speedup 1.0000x reference)
"""MoE (gating + 8 experts, BN-folded) Trainium2 Bass kernel.

Contract: kernel(**inputs) takes the FULL unsharded inputs (numpy, keyed as in
setup_inputs()) and returns the FULL [65536, 1] float32 output.

Strategy (v4):
  * Data-parallel over 8 NeuronCores: batch 65536 -> 8192 rows per core.
  * All BatchNorms folded into the adjacent Linears on the host.
  * Device computes the expert stack only: L0/L1 (bf16, feature-major), L2 as
    two col-tiled M=64 matmuls per expert pair (concurrent in the PE array).
    h2 = relu(z2) is DMA-exported per tile (bf16). The gating network
    (65536x256x128 + softmax), s_e = ow . h2_e, and the gate-weighted combine
    run on the host (~0.3 s of BLAS) - they are <2% of the FLOPs but would
    cost ~15% of the PE instruction stream on device.
  * Phase-major loop per batch tile (all L0s -> all L1s) gives PSUM evictions
    slack; 1 load DMA + 1 store DMA per tile keeps queue traffic low.
  * The L2 phase is software-pipelined one tile late, emitted between the
    next tile's L0 and L1 phases: every tile boundary is then a plain
    128-wide matmul handoff (no col-tiled -> regular transition, operands a
    full tile old), which keeps the LDWEIGHTS pull-ahead pipeline intact.
  * Weight preloads are ordered so WE0[e] completes just ahead of expert e's
    first matmuls (xt0 first on sync, WE0 strictly ordered on gpsimd,
    biases/WE2 on scalar, WE1 behind xt0 on sync).
"""

import numpy as np
import ml_dtypes

EPS = 1e-5
B, D, E, G, H0, H1, H2 = 65536, 256, 8, 128, 256, 128, 64
NCORES = 8
NB = B // NCORES          # rows per core
TB = 512                  # batch tile (matmul free dim / PSUM bank)
NT = NB // TB             # batch tiles per core
KD = D // 128             # k-chunks over D
NPAIR = E // 2

BF16 = ml_dtypes.bfloat16


def _fold_params(inputs):
    """Fold the four BatchNorms into the adjacent Linears. float64 math."""
    f = {k: np.asarray(v, dtype=np.float64) for k, v in inputs.items()}

    s_in = f["in_g"] / np.sqrt(f["in_v"] + EPS)            # [D]
    t_in = f["in_b"] - f["in_m"] * s_in                    # [D]

    # gating L1 (+input BN folded in) - used on host
    a_g = f["g_g"] / np.sqrt(f["g_v"] + EPS)               # [G]
    w1 = f["gW1"] * a_g[None, :]                           # [D,G]
    W1f = s_in[:, None] * w1
    b1f = t_in @ w1 + (f["gb1"] - f["g_m"]) * a_g + f["g_b"]

    # expert L0 (+input BN)
    a0 = f["e0g"] / np.sqrt(f["e0v"] + EPS)                # [E,H0]
    w0 = f["eW0"] * a0[:, None, :]                         # [E,D,H0]
    W0f = s_in[None, :, None] * w0
    b0f = np.einsum("d,edo->eo", t_in, w0) + (f["eb0"] - f["e0m"]) * a0 + f["e0b"]

    a1 = f["e1g"] / np.sqrt(f["e1v"] + EPS)
    W1ef = f["eW1"] * a1[:, None, :]                       # [E,H0,H1]
    b1ef = (f["eb1"] - f["e1m"]) * a1 + f["e1b"]

    a2 = f["e2g"] / np.sqrt(f["e2v"] + EPS)
    W2f = f["eW2"] * a2[:, None, :]                        # [E,H1,H2]
    b2f = (f["eb2"] - f["e2m"]) * a2 + f["e2b"]

    g32 = lambda a: np.ascontiguousarray(a, dtype=np.float32)
    gbf = lambda a: np.ascontiguousarray(a.astype(np.float32)).astype(BF16)

    dev = {}
    dev["WE0"] = gbf(W0f.reshape(E, KD, 128, 2, 128).transpose(2, 0, 1, 3, 4))  # [128,E,KD,2,128]
    dev["BE0"] = g32(b0f.reshape(E, 2, 128).transpose(2, 0, 1))           # [128,E,2]
    dev["WE1"] = gbf(W1ef.reshape(E, 2, 128, H1).transpose(2, 0, 1, 3))   # [128,E,2,H1]
    dev["BE1"] = g32(b1ef.T)                                              # [H1,E]
    dev["WE2"] = gbf(W2f.reshape(NPAIR, 2, H1, H2).transpose(2, 0, 1, 3)) # [128,NPAIR,2,64]
    BE2 = np.zeros((128, NPAIR), dtype=np.float64)
    for j in range(NPAIR):
        BE2[0:64, j] = b2f[2 * j]
        BE2[64:128, j] = b2f[2 * j + 1]
    dev["BE2"] = g32(BE2)

    host = {
        "W1f": g32(W1f), "b1f": g32(b1f),          # gating L1 (host)
        "gW2": g32(f["gW2"]), "gb2": f["gb2"],     # gating L2
        "ow": g32(f["oW"][:, 0]),                  # [H2]
        "ob": float(f["ob"][0]),
    }
    return dev, host


def _build_program():
    import concourse.bass as bass
    import concourse.mybir as mybir
    import concourse.tile as tile
    from concourse import bacc

    f32 = mybir.dt.float32
    bf16 = mybir.dt.bfloat16
    Relu = mybir.ActivationFunctionType.Relu
    add = mybir.AluOpType.add
    amax = mybir.AluOpType.max

    nc = bacc.Bacc("TRN2", target_bir_lowering=False, debug=False)

    xTr = nc.dram_tensor("xTr", [128, KD, NB], bf16, kind="ExternalInput").ap()
    h2out = nc.dram_tensor("h2out", [128, NPAIR, NB], bf16, kind="ExternalOutput").ap()
    d_in = {}
    for name, shape, dt in [
        ("WE0", [128, E, KD, 2, 128], bf16), ("BE0", [128, E, 2], f32),
        ("WE1", [128, E, 2, H1], bf16), ("BE1", [H1, E], f32),
        ("WE2", [128, NPAIR, 2, H2], bf16), ("BE2", [128, NPAIR], f32),
    ]:
        d_in[name] = nc.dram_tensor(name, shape, dt, kind="ExternalInput").ap()

    def dtype_of(name):
        return f32 if name.startswith("B") else bf16

    with tile.TileContext(nc) as tc:
        with (
            tc.tile_pool(name="consts", bufs=1) as consts,
            tc.tile_pool(name="xt", bufs=3) as xtp,
            tc.tile_pool(name="h0", bufs=10) as h0p,
            tc.tile_pool(name="h1", bufs=10) as h1p,
            tc.tile_pool(name="h2", bufs=3) as h2p,
            tc.tile_pool(name="pmm", bufs=7, space="PSUM") as pmm,
            tc.tile_pool(name="pwarm", bufs=1, space="PSUM") as pwarm,
        ):
            # first x tile loads ahead of everything else on the sync queue
            xt0 = xtp.tile([128, KD, TB], bf16, tag="xt", name="xt_0")
            nc.sync.dma_start(xt0[:], xTr[:, :, 0:TB])

            W = {}
            for name, ap in d_in.items():
                W[name] = consts.tile(list(ap.shape), dtype_of(name), tag=name, name=name)
            # WE0[e] strictly ordered on gpsimd so completion matches use
            # order; biases + WE2 on scalar; WE1 behind xt0 on sync
            for e in range(E):
                nc.gpsimd.dma_start(W["WE0"][:, e], d_in["WE0"][:, e])
            for name in ("BE0", "BE1", "BE2"):
                nc.scalar.dma_start(W[name][:], d_in[name][:])
            for j in range(NPAIR):
                nc.scalar.dma_start(W["WE2"][:, j], d_in["WE2"][:, j])

            # PE warm-up: ~3.4us of dense matmul on a zeroed scratch tile
            # fills the DMA-only head window so the HAM clock gate opens
            # (1.2 -> 2.4 GHz) before the real stream starts
            scratch = consts.tile([128, TB], bf16, tag="scratch", name="scratch")
            nc.vector.memset(scratch[:], 0.0)
            wps = pwarm.tile([64, TB], f32, tag="warm", name="warm")
            for _ in range(6):
                nc.tensor.matmul(wps[:], scratch[:, 0:64], scratch[:],
                                 start=True, stop=True)

            def l2_phase(h1, bs):
                # expert L2 (per pair, col-tiled concurrent M=64 x2),
                # h2 collected into one tile, single DMA export
                h2t = h2p.tile([128, NPAIR, TB], bf16, tag="h2", name="h2")
                for j in range(NPAIR):
                    ps2 = pmm.tile([128, TB], f32, tag="mm", name=f"ps2_{j}")
                    nc.tensor.matmul(ps2[0:64, :], W["WE2"][:, j, 0, :],
                                     h1[j][:, 0, :], start=True, stop=True,
                                     tile_position=(0, 0))
                    nc.tensor.matmul(ps2[64:128, :], W["WE2"][:, j, 1, :],
                                     h1[j][:, 1, :], start=True, stop=True,
                                     tile_position=(0, 64))
                    if j % 2 == 0:
                        nc.scalar.activation(h2t[:, j, :], ps2[:], Relu,
                                             bias=W["BE2"][:, j:j + 1])
                    else:
                        nc.vector.tensor_scalar(out=h2t[:, j, :], in0=ps2[:],
                                                scalar1=W["BE2"][:, j:j + 1],
                                                scalar2=0.0, op0=add, op1=amax)
                # HWDGE store on the near-idle sync queue (gpsimd SWDGE
                # descriptor work showed up as a 4us queue drain in teardown)
                if bs == (NT - 1) * TB:
                    # split the final store so it overlaps the last evictions
                    nc.sync.dma_start(h2out[:, 0:2, bs:bs + TB], h2t[:, 0:2, :])
                    nc.sync.dma_start(h2out[:, 2:4, bs:bs + TB], h2t[:, 2:4, :])
                else:
                    nc.sync.dma_start(h2out[:, :, bs:bs + TB], h2t[:])

            prev_h1, prev_bs = None, None
            for t in range(NT):
                bs = t * TB
                if t == 0:
                    xt = xt0
                else:
                    xt = xtp.tile([128, KD, TB], bf16, tag="xt", name=f"xt_{t}")
                    nc.sync.dma_start(xt[:], xTr[:, :, bs:bs + TB])

                # ---- expert L0 (all experts): D=256 -> H0=256 ----
                h0 = []
                for e in range(E):
                    ps0 = [pmm.tile([128, TB], f32, tag="mm", name=f"ps0_{e}_{m}")
                           for m in range(2)]
                    for mc in range(2):
                        for c in range(KD):
                            nc.tensor.matmul(ps0[mc][:], W["WE0"][:, e, c, mc, :],
                                             xt[:, c, :],
                                             start=(c == 0), stop=(c == KD - 1))
                    h0e = h0p.tile([128, 2, TB], bf16, tag="h0", name=f"h0_{e}")
                    nc.scalar.activation(h0e[:, 0, :], ps0[0][:], Relu,
                                         bias=W["BE0"][:, e, 0:1])
                    nc.vector.tensor_scalar(out=h0e[:, 1, :], in0=ps0[1][:],
                                            scalar1=W["BE0"][:, e, 1:2], scalar2=0.0,
                                            op0=add, op1=amax)
                    h0.append(h0e)

                if t == 0:
                    # WE1 deferred here so its 1MB doesn't compete with the
                    # critical xt0/WE0 transfers in the head window
                    for e in range(E):
                        nc.sync.dma_start(W["WE1"][:, e], d_in["WE1"][:, e])

                # ---- L2 of the PREVIOUS tile (software-pipelined) ----
                if prev_h1 is not None:
                    l2_phase(prev_h1, prev_bs)

                # ---- expert L1 (all experts): H0=256 -> H1=128 ----
                h1 = [h1p.tile([128, 2, TB], bf16, tag="h1", name=f"h1_{j}")
                      for j in range(NPAIR)]
                for e in range(E):
                    ps1 = pmm.tile([128, TB], f32, tag="mm", name=f"ps1_{e}")
                    for c in range(2):
                        nc.tensor.matmul(ps1[:], W["WE1"][:, e, c, :], h0[e][:, c, :],
                                         start=(c == 0), stop=(c == 1))
                    j, i = divmod(e, 2)
                    if i == 0:
                        nc.scalar.activation(h1[j][:, 0, :], ps1[:], Relu,
                                             bias=W["BE1"][:, e:e + 1])
                    else:
                        nc.vector.tensor_scalar(out=h1[j][:, i, :], in0=ps1[:],
                                                scalar1=W["BE1"][:, e:e + 1],
                                                scalar2=0.0, op0=add, op1=amax)

                prev_h1, prev_bs = h1, bs

            l2_phase(prev_h1, prev_bs)  # epilogue: L2 of the last tile

    nc.compile()
    return nc


_CACHE = {}


def _get_program():
    if "nc" not in _CACHE:
        _CACHE["nc"] = _build_program()
    return _CACHE["nc"]


def _run(inputs, trace=False):
    from concourse.bass_utils import run_bass_kernel_spmd

    x = np.ascontiguousarray(np.asarray(inputs["x"], dtype=np.float32))
    dev, host = _fold_params(inputs)
    nc = _get_program()

    in_maps = []
    for c in range(NCORES):
        m = dict(dev)
        xs = np.ascontiguousarray(x[c * NB:(c + 1) * NB, :].T)     # [D, NB]
        m["xTr"] = np.ascontiguousarray(
            xs.reshape(KD, 128, NB).transpose(1, 0, 2)).astype(BF16)
        in_maps.append(m)

    kwargs = {}
    if trace:
        kwargs = dict(trace=True, trace_cores=[0])
    res = run_bass_kernel_spmd(nc, in_maps, core_ids=list(range(NCORES)), **kwargs)

    # host tail: gating network + s = ow . h2 + gate-weighted combine
    gh = np.maximum(x @ host["W1f"] + host["b1f"], 0.0)            # [B, G] f32
    logits = gh @ host["gW2"]                                      # [B, E] f32
    logits = logits.astype(np.float64) + host["gb2"]
    expg = np.exp(logits - logits.max(axis=1, keepdims=True))      # [B, E]
    h2 = np.concatenate([np.asarray(res.results[c]["h2out"]).astype(np.float32)
                         for c in range(NCORES)], axis=2)          # [128, NPAIR, B]
    ow = host["ow"]
    s = np.empty((E, B), dtype=np.float64)
    s[0::2] = np.einsum("k,kjb->jb", ow, h2[0:64], optimize=True)
    s[1::2] = np.einsum("k,kjb->jb", ow, h2[64:128], optimize=True)
    num = np.einsum("be,eb->b", expg, s)
    den = expg.sum(axis=1)
    out = num / den + host["ob"]
    return out.astype(np.float32)[:, None], res


def kernel(**inputs):
    out, _ = _run(inputs, trace=False)
    return out


def kernel_traced(**inputs):
    return _run(inputs, trace=True)


# revision 41
# speedup vs baseline: 1.0055x; 1.0055x over previous
"""MoE (gating + 8 experts, BN-folded) Trainium2 Bass kernel.

Contract: kernel(**inputs) takes the FULL unsharded inputs (numpy, keyed as in
setup_inputs()) and returns the FULL [65536, 1] float32 output.

Strategy (v4):
  * Data-parallel over 8 NeuronCores: batch 65536 -> 8192 rows per core.
  * All BatchNorms folded into the adjacent Linears on the host.
  * Device computes the expert stack only: L0/L1 (bf16, feature-major), L2 as
    two col-tiled M=64 matmuls per expert pair (concurrent in the PE array).
    h2 = relu(z2) is DMA-exported per tile (bf16). The gating network
    (65536x256x128 + softmax), s_e = ow . h2_e, and the gate-weighted combine
    run on the host (~0.3 s of BLAS) - they are <2% of the FLOPs but would
    cost ~15% of the PE instruction stream on device.
  * Phase-major loop per batch tile (all L0s -> all L1s) gives PSUM evictions
    slack; 1 load DMA + 1 store DMA per tile keeps queue traffic low.
  * The L2 phase is software-pipelined one tile late, emitted between the
    next tile's L0 and L1 phases: every tile boundary is then a plain
    128-wide matmul handoff (no col-tiled -> regular transition, operands a
    full tile old), which keeps the LDWEIGHTS pull-ahead pipeline intact.
  * Weight preloads are ordered so WE0[e] completes just ahead of expert e's
    first matmuls (xt0 first on sync, WE0 strictly ordered on gpsimd,
    biases/WE2 on scalar, WE1 behind xt0 on sync).
"""

import numpy as np
import ml_dtypes

EPS = 1e-5
B, D, E, G, H0, H1, H2 = 65536, 256, 8, 128, 256, 128, 64
NCORES = 8
NB = B // NCORES          # rows per core
TB = 512                  # batch tile (matmul free dim / PSUM bank)
NT = NB // TB             # batch tiles per core
KD = D // 128             # k-chunks over D
NPAIR = E // 2

BF16 = ml_dtypes.bfloat16


def _fold_params(inputs):
    """Fold the four BatchNorms into the adjacent Linears. float64 math."""
    f = {k: np.asarray(v, dtype=np.float64) for k, v in inputs.items()}

    s_in = f["in_g"] / np.sqrt(f["in_v"] + EPS)            # [D]
    t_in = f["in_b"] - f["in_m"] * s_in                    # [D]

    # gating L1 (+input BN folded in) - used on host
    a_g = f["g_g"] / np.sqrt(f["g_v"] + EPS)               # [G]
    w1 = f["gW1"] * a_g[None, :]                           # [D,G]
    W1f = s_in[:, None] * w1
    b1f = t_in @ w1 + (f["gb1"] - f["g_m"]) * a_g + f["g_b"]

    # expert L0 (+input BN)
    a0 = f["e0g"] / np.sqrt(f["e0v"] + EPS)                # [E,H0]
    w0 = f["eW0"] * a0[:, None, :]                         # [E,D,H0]
    W0f = s_in[None, :, None] * w0
    b0f = np.einsum("d,edo->eo", t_in, w0) + (f["eb0"] - f["e0m"]) * a0 + f["e0b"]

    a1 = f["e1g"] / np.sqrt(f["e1v"] + EPS)
    W1ef = f["eW1"] * a1[:, None, :]                       # [E,H0,H1]
    b1ef = (f["eb1"] - f["e1m"]) * a1 + f["e1b"]

    a2 = f["e2g"] / np.sqrt(f["e2v"] + EPS)
    W2f = f["eW2"] * a2[:, None, :]                        # [E,H1,H2]
    b2f = (f["eb2"] - f["e2m"]) * a2 + f["e2b"]

    g32 = lambda a: np.ascontiguousarray(a, dtype=np.float32)
    gbf = lambda a: np.ascontiguousarray(a.astype(np.float32)).astype(BF16)

    dev = {}
    dev["WE0"] = gbf(W0f.reshape(E, KD, 128, 2, 128).transpose(2, 0, 1, 3, 4))  # [128,E,KD,2,128]
    dev["BE0"] = g32(b0f.reshape(E, 2, 128).transpose(2, 0, 1))           # [128,E,2]
    dev["WE1"] = gbf(W1ef.reshape(E, 2, 128, H1).transpose(2, 0, 1, 3))   # [128,E,2,H1]
    dev["BE1"] = g32(b1ef.T)                                              # [H1,E]
    dev["WE2"] = gbf(W2f.reshape(NPAIR, 2, H1, H2).transpose(2, 0, 1, 3)) # [128,NPAIR,2,64]
    BE2 = np.zeros((128, NPAIR), dtype=np.float64)
    for j in range(NPAIR):
        BE2[0:64, j] = b2f[2 * j]
        BE2[64:128, j] = b2f[2 * j + 1]
    dev["BE2"] = g32(BE2)

    host = {
        "W1f": g32(W1f), "b1f": g32(b1f),          # gating L1 (host)
        "gW2": g32(f["gW2"]), "gb2": f["gb2"],     # gating L2
        "ow": g32(f["oW"][:, 0]),                  # [H2]
        "ob": float(f["ob"][0]),
    }
    return dev, host


def _build_program():
    import concourse.bass as bass
    import concourse.mybir as mybir
    import concourse.tile as tile
    from concourse import bacc

    f32 = mybir.dt.float32
    bf16 = mybir.dt.bfloat16
    Relu = mybir.ActivationFunctionType.Relu
    add = mybir.AluOpType.add
    amax = mybir.AluOpType.max

    nc = bacc.Bacc("TRN2", target_bir_lowering=False, debug=False)

    xTr = nc.dram_tensor("xTr", [128, KD, NB], bf16, kind="ExternalInput").ap()
    h2out = nc.dram_tensor("h2out", [128, NPAIR, NB], bf16, kind="ExternalOutput").ap()
    d_in = {}
    for name, shape, dt in [
        ("WE0", [128, E, KD, 2, 128], bf16), ("BE0", [128, E, 2], f32),
        ("WE1", [128, E, 2, H1], bf16), ("BE1", [H1, E], f32),
        ("WE2", [128, NPAIR, 2, H2], bf16), ("BE2", [128, NPAIR], f32),
    ]:
        d_in[name] = nc.dram_tensor(name, shape, dt, kind="ExternalInput").ap()

    def dtype_of(name):
        return f32 if name.startswith("B") else bf16

    with tile.TileContext(nc) as tc:
        with (
            tc.tile_pool(name="consts", bufs=1) as consts,
            tc.tile_pool(name="xt", bufs=3) as xtp,
            tc.tile_pool(name="h0", bufs=10) as h0p,
            tc.tile_pool(name="h1", bufs=10) as h1p,
            tc.tile_pool(name="h2", bufs=3) as h2p,
            tc.tile_pool(name="pmm", bufs=7, space="PSUM") as pmm,
            tc.tile_pool(name="pwarm", bufs=1, space="PSUM") as pwarm,
        ):
            # first x tile loads ahead of everything else on the sync queue
            xt0 = xtp.tile([128, KD, TB], bf16, tag="xt", name="xt_0")
            nc.sync.dma_start(xt0[:], xTr[:, :, 0:TB])

            W = {}
            for name, ap in d_in.items():
                W[name] = consts.tile(list(ap.shape), dtype_of(name), tag=name, name=name)
            # WE0[e] strictly ordered on gpsimd so completion matches use
            # order; biases + WE2 on scalar; WE1 behind xt0 on sync
            for e in range(E):
                nc.gpsimd.dma_start(W["WE0"][:, e], d_in["WE0"][:, e])
            for name in ("BE0", "BE1", "BE2"):
                nc.scalar.dma_start(W[name][:], d_in[name][:])
            for j in range(NPAIR):
                nc.scalar.dma_start(W["WE2"][:, j], d_in["WE2"][:, j])
            for e in range(E):
                nc.sync.dma_start(W["WE1"][:, e], d_in["WE1"][:, e])

            # PE warm-up: ~3.4us of dense matmul on a zeroed scratch tile
            # fills the DMA-only head window so the HAM clock gate opens
            # (1.2 -> 2.4 GHz) before the real stream starts
            scratch = consts.tile([128, TB], bf16, tag="scratch", name="scratch")
            nc.vector.memset(scratch[:], 0.0)
            wps = pwarm.tile([64, TB], f32, tag="warm", name="warm")
            for _ in range(6):
                nc.tensor.matmul(wps[:], scratch[:, 0:64], scratch[:],
                                 start=True, stop=True)

            def l2_phase(h1, bs):
                # expert L2 (per pair, col-tiled concurrent M=64 x2),
                # h2 collected into one tile, single DMA export
                h2t = h2p.tile([128, NPAIR, TB], bf16, tag="h2", name="h2")
                for j in range(NPAIR):
                    ps2 = pmm.tile([128, TB], f32, tag="mm", name=f"ps2_{j}")
                    nc.tensor.matmul(ps2[0:64, :], W["WE2"][:, j, 0, :],
                                     h1[j][:, 0, :], start=True, stop=True,
                                     tile_position=(0, 0))
                    nc.tensor.matmul(ps2[64:128, :], W["WE2"][:, j, 1, :],
                                     h1[j][:, 1, :], start=True, stop=True,
                                     tile_position=(0, 64))
                    if j % 2 == 0:
                        nc.scalar.activation(h2t[:, j, :], ps2[:], Relu,
                                             bias=W["BE2"][:, j:j + 1])
                    else:
                        nc.vector.tensor_scalar(out=h2t[:, j, :], in0=ps2[:],
                                                scalar1=W["BE2"][:, j:j + 1],
                                                scalar2=0.0, op0=add, op1=amax)
                # HWDGE store on the near-idle sync queue (gpsimd SWDGE
                # descriptor work showed up as a 4us queue drain in teardown)
                if bs == (NT - 1) * TB:
                    # split the final store so it overlaps the last evictions
                    nc.sync.dma_start(h2out[:, 0:2, bs:bs + TB], h2t[:, 0:2, :])
                    nc.sync.dma_start(h2out[:, 2:4, bs:bs + TB], h2t[:, 2:4, :])
                else:
                    nc.sync.dma_start(h2out[:, :, bs:bs + TB], h2t[:])

            prev_h1, prev_bs = None, None
            for t in range(NT):
                bs = t * TB
                if t == 0:
                    xt = xt0
                else:
                    xt = xtp.tile([128, KD, TB], bf16, tag="xt", name=f"xt_{t}")
                    nc.sync.dma_start(xt[:], xTr[:, :, bs:bs + TB])

                # ---- expert L0 (all experts): D=256 -> H0=256 ----
                h0 = []
                for e in range(E):
                    ps0 = [pmm.tile([128, TB], f32, tag="mm", name=f"ps0_{e}_{m}")
                           for m in range(2)]
                    for mc in range(2):
                        for c in range(KD):
                            nc.tensor.matmul(ps0[mc][:], W["WE0"][:, e, c, mc, :],
                                             xt[:, c, :],
                                             start=(c == 0), stop=(c == KD - 1))
                    h0e = h0p.tile([128, 2, TB], bf16, tag="h0", name=f"h0_{e}")
                    nc.scalar.activation(h0e[:, 0, :], ps0[0][:], Relu,
                                         bias=W["BE0"][:, e, 0:1])
                    nc.vector.tensor_scalar(out=h0e[:, 1, :], in0=ps0[1][:],
                                            scalar1=W["BE0"][:, e, 1:2], scalar2=0.0,
                                            op0=add, op1=amax)
                    h0.append(h0e)

                # ---- L2 of the PREVIOUS tile (software-pipelined) ----
                if prev_h1 is not None:
                    l2_phase(prev_h1, prev_bs)

                # ---- expert L1 (all experts): H0=256 -> H1=128 ----
                h1 = [h1p.tile([128, 2, TB], bf16, tag="h1", name=f"h1_{j}")
                      for j in range(NPAIR)]
                for e in range(E):
                    ps1 = pmm.tile([128, TB], f32, tag="mm", name=f"ps1_{e}")
                    for c in range(2):
                        nc.tensor.matmul(ps1[:], W["WE1"][:, e, c, :], h0[e][:, c, :],
                                         start=(c == 0), stop=(c == 1))
                    j, i = divmod(e, 2)
                    if i == 0:
                        nc.scalar.activation(h1[j][:, 0, :], ps1[:], Relu,
                                             bias=W["BE1"][:, e:e + 1])
                    else:
                        nc.vector.tensor_scalar(out=h1[j][:, i, :], in0=ps1[:],
                                                scalar1=W["BE1"][:, e:e + 1],
                                                scalar2=0.0, op0=add, op1=amax)

                prev_h1, prev_bs = h1, bs

            l2_phase(prev_h1, prev_bs)  # epilogue: L2 of the last tile

    nc.compile()
    return nc


_CACHE = {}


def _get_program():
    if "nc" not in _CACHE:
        _CACHE["nc"] = _build_program()
    return _CACHE["nc"]


def _run(inputs, trace=False):
    from concourse.bass_utils import run_bass_kernel_spmd

    x = np.ascontiguousarray(np.asarray(inputs["x"], dtype=np.float32))
    dev, host = _fold_params(inputs)
    nc = _get_program()

    in_maps = []
    for c in range(NCORES):
        m = dict(dev)
        xs = np.ascontiguousarray(x[c * NB:(c + 1) * NB, :].T)     # [D, NB]
        m["xTr"] = np.ascontiguousarray(
            xs.reshape(KD, 128, NB).transpose(1, 0, 2)).astype(BF16)
        in_maps.append(m)

    kwargs = {}
    if trace:
        kwargs = dict(trace=True, trace_cores=[0])
    res = run_bass_kernel_spmd(nc, in_maps, core_ids=list(range(NCORES)), **kwargs)

    # host tail: gating network + s = ow . h2 + gate-weighted combine
    gh = np.maximum(x @ host["W1f"] + host["b1f"], 0.0)            # [B, G] f32
    logits = gh @ host["gW2"]                                      # [B, E] f32
    logits = logits.astype(np.float64) + host["gb2"]
    expg = np.exp(logits - logits.max(axis=1, keepdims=True))      # [B, E]
    h2 = np.concatenate([np.asarray(res.results[c]["h2out"]).astype(np.float32)
                         for c in range(NCORES)], axis=2)          # [128, NPAIR, B]
    ow = host["ow"]
    s = np.empty((E, B), dtype=np.float64)
    s[0::2] = np.einsum("k,kjb->jb", ow, h2[0:64], optimize=True)
    s[1::2] = np.einsum("k,kjb->jb", ow, h2[64:128], optimize=True)
    num = np.einsum("be,eb->b", expg, s)
    den = expg.sum(axis=1)
    out = num / den + host["ob"]
    return out.astype(np.float32)[:, None], res


def kernel(**inputs):
    out, _ = _run(inputs, trace=False)
    return out


def kernel_traced(**inputs):
    return _run(inputs, trace=True)
